# revision 16
# baseline (speedup 1.0000x reference)
"""MoE-routed multi-head attention kernel for 8 Trainium2 NeuronCores.

Problem shape (hardcoded):
  query/key/value: [4, 2048, 512] f32
  Wg [512,8], Wk/Wv [512,64], Wq [8,512,64], Wo [8,64,512], biases.
  TOP_K=2 routed experts act as the two attention heads.

Sharding: core c = 2*b + h handles batch b, query-half h (1024 query tokens),
with the full 2048 keys/values of batch b. All compute stays on device; the
host only slices/transposes/casts inputs and concatenates outputs.

Numerics: matmul operands are bf16 (fp32 PSUM accumulation); the router is
computed exactly via a 3-term bf16 hi/lo split so top-2 expert selection
matches fp32; softmax normalization, gates and combine scalars are fp32.

Layout notes:
 - khT and qselT are stacked per head on partitions 0-63 / 64-127 so the two
   heads' attention-logit matmuls run concurrently in separate PE row groups.
 - q_all / masks / c use (d, e)-major columns so expert reduction is over the
   contiguous innermost axis.
"""

import numpy as np

import concourse.bass as bass
import concourse.mybir as mybir
import concourse.tile as tile
from concourse import bacc
from concourse import bass_utils
from concourse.masks import make_identity

P = 128
D = 512          # d_model
T = 2048         # kv tokens per core (full batch)
NQ = 1024        # query tokens per core
E = 8            # experts
DK = 64          # head dim
DC = D // P      # 4 contraction chunks
NKC = T // P     # 16 key chunks
NQT = NQ // P    # 8 query tiles
VW = DK + 1      # vh columns + ones column (denominator trick)
HD = D           # half of NQ (phase-C column granularity)

FP = mybir.dt.float32
U32 = mybir.dt.uint32
BF = mybir.dt.bfloat16
AF = mybir.ActivationFunctionType
OP = mybir.AluOpType
AX = mybir.AxisListType

DEBUG = False


def _emit(nc, tc, ctx):
    const = ctx.enter_context(tc.tile_pool(name="const", bufs=1))
    persist = ctx.enter_context(tc.tile_pool(name="persist", bufs=1))
    work = ctx.enter_context(tc.tile_pool(name="work", bufs=3))
    kvwork = ctx.enter_context(tc.tile_pool(name="kvwork", bufs=2))
    expp = ctx.enter_context(tc.tile_pool(name="expp", bufs=4))
    ps_log = ctx.enter_context(tc.tile_pool(name="ps_log", bufs=4, space="PSUM"))
    ps_att = ctx.enter_context(tc.tile_pool(name="ps_att", bufs=2, space="PSUM"))
    ps_sm = ctx.enter_context(tc.tile_pool(name="ps_sm", bufs=2, space="PSUM"))

    dram = {}
    for name, shape, dt in [
        ("qT_hi", [D, NQ], BF), ("qT_lo", [D, NQ], BF),
        ("kT", [D, T], BF), ("vT", [D, T], BF),
        ("Wg_hi", [D, E], BF), ("Wg_lo", [D, E], BF),
        ("Wk2", [D, P], BF), ("Wv", [D, DK], BF),
        ("Wq_f", [D, DK * E], BF), ("Wo_f", [DK * E, D], BF),
        ("bkT2", [P, 1], FP), ("bvT", [DK, 1], FP),
        ("bq_f", [1, DK * E], BF), ("bo", [E, D], BF),
        ("ones1", [1, P], BF), ("onescol", [P, NKC], BF),
    ]:
        dram[name] = nc.dram_tensor(name, shape, dt, kind="ExternalInput").ap()
    out_d = nc.dram_tensor("out", [NQ, D], FP, kind="ExternalOutput").ap()
    if DEBUG:
        dbg_lg = nc.dram_tensor("dbg_lg", [NQ, E], FP, kind="ExternalOutput").ap()
        dbg_i = nc.dram_tensor("dbg_i", [NQ, E], U32, kind="ExternalOutput").ap()
        dbg_g = nc.dram_tensor("dbg_g", [NQ, 2], FP, kind="ExternalOutput").ap()

    # ---- constants ----
    ident = const.tile([P, P], FP, tag="ident")
    make_identity(nc, ident[:])
    ident_b = const.tile([P, P], BF, tag="ident_b")
    make_identity(nc, ident_b[:])
    iota_e = const.tile([P, DK * E], BF, tag="iota_e")  # expert idx, (d e) cols
    nc.gpsimd.iota(iota_e[:].rearrange("p (d e) -> p d e", e=E),
                   pattern=[[0, DK], [1, E]], channel_multiplier=0,
                   allow_small_or_imprecise_dtypes=True)
    iota8 = const.tile([P, E], FP, tag="iota8")
    nc.gpsimd.iota(iota8[:], pattern=[[1, E]], channel_multiplier=0,
                   allow_small_or_imprecise_dtypes=True)
    ones_row = const.tile([1, P], BF, tag="ones_row")
    nc.sync.dma_start(ones_row[:], dram["ones1"])

    # ---- weights (all bf16) ----
    w = {}
    for name, cols in [("Wk2", P), ("Wv", DK), ("Wq_f", DK * E), ("Wo_f", D),
                       ("Wg_hi", E), ("Wg_lo", E)]:
        for dc in range(DC):
            t = const.tile([P, cols], BF, tag=f"{name}{dc}", name=f"w_{name}{dc}")
            nc.sync.dma_start(t[:], dram[name][dc * P:(dc + 1) * P, :])
            w[(name, dc)] = t
    bkT2 = const.tile([P, 1], FP, tag="bkT2")
    nc.sync.dma_start(bkT2[:], dram["bkT2"])
    bvT = const.tile([DK, 1], FP, tag="bvT")
    nc.sync.dma_start(bvT[:], dram["bvT"])
    bq_f = const.tile([1, DK * E], BF, tag="bq_f")
    nc.sync.dma_start(bq_f[:], dram["bq_f"])
    bo = const.tile([E, D], BF, tag="bo")
    nc.sync.dma_start(bo[:], dram["bo"])

    # ---- query (resident, transposed, bf16 hi/lo) ----
    qTsp = {}
    for name in ["qT_hi", "qT_lo"]:
        for dc in range(DC):
            t = persist.tile([P, NQ], BF, tag=f"{name}{dc}", name=f"{name}{dc}")
            nc.sync.dma_start(t[:], dram[name][dc * P:(dc + 1) * P, :])
            qTsp[(name, dc)] = t

    # ---- router logits, exact: routerT[8, NQ] = Wg_hi/lo.T @ q_hi/lo ----
    routerT = persist.tile([E, NQ], FP, tag="routerT")
    for half in range(2):
        hs = slice(half * HD, (half + 1) * HD)
        ps_rt = ps_sm.tile([E, HD], FP, tag="ps")
        first = True
        for dc in range(DC):
            for lname, rname in [("Wg_hi", "qT_hi"), ("Wg_hi", "qT_lo"),
                                 ("Wg_lo", "qT_hi")]:
                nc.tensor.matmul(ps_rt[:], w[(lname, dc)][:],
                                 qTsp[(rname, dc)][:, hs], start=first,
                                 stop=(dc == DC - 1 and lname == "Wg_lo"))
                first = False
        nc.vector.tensor_copy(routerT[:, hs], ps_rt[:])

    # ---- Phase A: khT2 [128, T] (both heads' copies), vh_aug [128, 16*65] ----
    khT2 = persist.tile([P, T], BF, tag="khT2")
    vhT = persist.tile([DK, T], BF, tag="vhT")
    for ncH in range(4):
        slab = [kvwork.tile([P, HD], BF, tag=f"kslab{dc}", name=f"kslab{dc}")
                for dc in range(DC)]
        for dc in range(DC):
            nc.sync.dma_start(
                slab[dc][:], dram["kT"][dc * P:(dc + 1) * P, ncH * HD:(ncH + 1) * HD])
        ps = ps_sm.tile([P, HD], FP, tag="ps")
        for dc in range(DC):
            nc.tensor.matmul(ps[:], w[("Wk2", dc)][:], slab[dc][:],
                             start=(dc == 0), stop=(dc == DC - 1))
        nc.vector.tensor_scalar(khT2[:, ncH * HD:(ncH + 1) * HD], ps[:],
                                bkT2[:], None, op0=OP.add)
    for ncH in range(4):
        slab = [kvwork.tile([P, HD], BF, tag=f"vslab{dc}", name=f"vslab{dc}")
                for dc in range(DC)]
        for dc in range(DC):
            nc.sync.dma_start(
                slab[dc][:], dram["vT"][dc * P:(dc + 1) * P, ncH * HD:(ncH + 1) * HD])
        ps = ps_sm.tile([DK, HD], FP, tag="ps")
        for dc in range(DC):
            nc.tensor.matmul(ps[:], w[("Wv", dc)][:], slab[dc][:],
                             start=(dc == 0), stop=(dc == DC - 1))
        nc.vector.tensor_scalar(vhT[:, ncH * HD:(ncH + 1) * HD], ps[:],
                                bvT[:], None, op0=OP.add)

    vh_aug = persist.tile([P, NKC * VW], BF, tag="vh_aug")
    nc.sync.dma_start(vh_aug[:].rearrange("p (c w) -> p c w", w=VW)[:, :, DK],
                      dram["onescol"])
    for kc in range(NKC):
        ps = ps_sm.tile([P, P], BF, tag="ps")
        nc.tensor.matmul(ps[:, :DK], vhT[:, kc * P:(kc + 1) * P],
                         ident_b[:DK, :DK], is_transpose=True)
        nc.vector.tensor_copy(vh_aug[:, kc * VW:kc * VW + DK], ps[:, :DK])

    # ---- Phase B-route: top-2 selection + selected query projections ----
    qselT2 = persist.tile([P, NQ], BF, tag="qselT2")  # h0 rows 0-63, h1 rows 64-127
    g_sb = [persist.tile([P, NQT], FP, tag=f"g{h}", name=f"g{h}") for h in range(2)]
    lg8s, m8s, if8s, mreps = {}, {}, {}, {}
    for qt in range(NQT):
        qs = slice(qt * P, (qt + 1) * P)
        # all-expert query projection (q_hi only) + bias
        ps_qa = ps_sm.tile([P, DK * E], FP, tag="ps")
        for dc in range(DC):
            nc.tensor.matmul(ps_qa[:], qTsp[("qT_hi", dc)][:, qs],
                             w[("Wq_f", dc)][:], start=(dc == 0), stop=False)
        nc.tensor.matmul(ps_qa[:], ones_row[:], bq_f[:], start=False, stop=True)
        qa_b = work.tile([P, DK * E], BF, tag="qa_b")
        nc.scalar.activation(qa_b[:], ps_qa[:], AF.Copy)

        # router logits for this tile: transpose routerT slice -> [P, E]
        ps_r = ps_sm.tile([P, E], FP, tag="ps")
        nc.tensor.matmul(ps_r[:], routerT[:, qs], ident[:E, :E], is_transpose=True)
        lg8 = persist.tile([P, E], FP, tag=f"lg8_{qt}", name=f"lg8_{qt}")
        nc.vector.tensor_copy(lg8[:], ps_r[:])
        lg8s[qt] = lg8
        m8 = persist.tile([P, E], FP, tag=f"m8_{qt}", name=f"m8_{qt}")
        nc.vector.max(out=m8[:], in_=lg8[:])
        m8s[qt] = m8
        i8 = work.tile([P, E], U32, tag="i8")
        nc.vector.max_index(i8[:], m8[:], lg8[:])
        if8 = persist.tile([P, 2], FP, tag=f"if8_{qt}", name=f"if8_{qt}")
        nc.vector.tensor_copy(if8[:], i8[:, 0:2])
        if8s[qt] = if8
        if DEBUG:
            nc.sync.dma_start(dbg_lg[qs, :], lg8[:])
            nc.sync.dma_start(dbg_i[qs, :], i8[:])
        # per-head expert masks ((d e) layout), select, reduce; both heads'
        # selected queries land side-by-side in qsel2 and transpose in one shot
        qsel2 = work.tile([P, P], FP, tag="qsel2")
        for h in range(2):
            eng = nc.vector if h == 0 else nc.gpsimd
            mrep = persist.tile([P, DK * E], BF, tag=f"mrep{qt}_{h}",
                                name=f"mrep{qt}_{h}")
            eng.tensor_scalar(mrep[:], iota_e[:], if8[:, h:h + 1], None,
                              op0=OP.is_equal)
            mreps[(qt, h)] = mrep
            u = work.tile([P, DK * E], BF, tag=f"u{h}", name=f"u{h}")
            eng.tensor_tensor(u[:], qa_b[:], mrep[:], op=OP.mult)
            nc.vector.reduce_sum(qsel2[:, h * DK:(h + 1) * DK],
                                 u[:].rearrange("p (d e) -> p d e", e=E),
                                 axis=AX.X)
        ps_t = ps_sm.tile([P, P], FP, tag="ps")
        nc.tensor.matmul(ps_t[:], qsel2[:], ident[:], is_transpose=True)
        nc.vector.tensor_copy(qselT2[:, qs], ps_t[:])

    # ---- Phase B-gate: softmax gates + combine weights (needed in D) ----
    combT = persist.tile([E, NQ], BF, tag="combT")
    for qt in range(NQT):
        qs = slice(qt * P, (qt + 1) * P)
        e8 = work.tile([P, E], FP, tag="e8")
        nc.scalar.activation(e8[:], lg8s[qt][:], AF.Exp)
        gtop = work.tile([P, 2], FP, tag="gtop")
        nc.scalar.activation(gtop[:], m8s[qt][:, 0:2], AF.Exp)
        ssum = work.tile([P, 1], FP, tag="ssum")
        nc.vector.reduce_sum(ssum[:], e8[:], axis=AX.X)
        srec = work.tile([P, 1], FP, tag="srec")
        nc.vector.reciprocal(srec[:], ssum[:])
        for h in range(2):
            nc.vector.tensor_tensor(g_sb[h][:, qt:qt + 1], gtop[:, h:h + 1],
                                    srec[:], op=OP.mult)
        if DEBUG:
            for h in range(2):
                nc.sync.dma_start(dbg_g[qs, h:h + 1], g_sb[h][:, qt:qt + 1])
        comb8 = work.tile([P, E], FP, tag="comb8")
        tmp8 = work.tile([P, E], FP, tag="tmp8")
        nc.vector.scalar_tensor_tensor(
            comb8[:], iota8[:], if8s[qt][:, 0:1],
            g_sb[0][:, qt:qt + 1].to_broadcast((P, E)), op0=OP.is_equal, op1=OP.mult)
        nc.vector.scalar_tensor_tensor(
            tmp8[:], iota8[:], if8s[qt][:, 1:2],
            g_sb[1][:, qt:qt + 1].to_broadcast((P, E)), op0=OP.is_equal, op1=OP.mult)
        nc.vector.tensor_tensor(comb8[:], comb8[:], tmp8[:], op=OP.add)
        ps_c = ps_sm.tile([E, P], FP, tag="ps")
        nc.tensor.matmul(ps_c[:], comb8[:], ident[:], is_transpose=True)
        nc.vector.tensor_copy(combT[:, qs], ps_c[:])

    # ---- Phase C: attention, halves outer, heads packed in PE row groups ----
    attnT = [persist.tile([VW, NQ], FP, tag=f"attnT{h}", name=f"attnT{h}")
             for h in range(2)]
    for half in range(2):
        hs = slice(half * HD, (half + 1) * HD)
        ps_a = [ps_att.tile([VW, HD], FP, tag="ps_a", name=f"ps_a{h}")
                for h in range(2)]
        pending = None  # software pipeline: attn MMs trail logits by one chunk
        for kc in range(NKC):
            ps_l = [ps_log.tile([P, HD], FP, tag="ps_l", name=f"ps_l{h}")
                    for h in range(2)]
            for h in range(2):
                rg = slice(h * DK, (h + 1) * DK)
                nc.tensor.matmul(ps_l[h][:], khT2[rg, kc * P:(kc + 1) * P],
                                 qselT2[rg, hs], start=True, stop=True)
            if pending is not None:
                pkc, pex = pending
                for h in range(2):
                    nc.tensor.matmul(ps_a[h][:], vh_aug[:, pkc * VW:(pkc + 1) * VW],
                                     pex[h][:], start=(pkc == 0),
                                     stop=(pkc == NKC - 1), skip_group_check=True)
            ex = [expp.tile([P, HD], BF, tag="ex", name=f"ex{h}") for h in range(2)]
            for h in range(2):
                nc.scalar.activation(ex[h][:], ps_l[h][:], AF.Exp, scale=0.125)
            pending = (kc, ex)
        pkc, pex = pending
        for h in range(2):
            nc.tensor.matmul(ps_a[h][:], vh_aug[:, pkc * VW:(pkc + 1) * VW],
                             pex[h][:], start=(pkc == 0), stop=(pkc == NKC - 1),
                             skip_group_check=True)
        for h in range(2):
            nc.vector.tensor_copy(attnT[h][:, hs], ps_a[h][:])

    # ---- Phase D: combine + output projection, per q-tile ----
    for qt in range(NQT):
        qs = slice(qt * P, (qt + 1) * P)
        at = []
        sc = []
        for h in range(2):
            ps_t = ps_sm.tile([P, VW], FP, tag="ps")
            nc.tensor.matmul(ps_t[:], attnT[h][:, qs], ident[:VW, :VW],
                             is_transpose=True)
            a = work.tile([P, VW], FP, tag="attn_tr")
            nc.vector.tensor_copy(a[:], ps_t[:])
            at.append(a)
            dinv = work.tile([P, 1], FP, tag="dinv")
            nc.vector.reciprocal(dinv[:], a[:, DK:DK + 1])
            s = work.tile([P, 1], FP, tag="s")
            nc.vector.tensor_tensor(s[:], g_sb[h][:, qt:qt + 1], dinv[:], op=OP.mult)
            sc.append(s)
        # c = sum_h (attn_h bcast * s_h) * mask_rep_h  ((d e) cols, disjoint)
        ch = []
        for h in range(2):
            eng = nc.vector
            c = work.tile([P, DK * E], BF, tag=f"c{h}", name=f"c{h}")
            eng.scalar_tensor_tensor(
                c[:].rearrange("p (d e) -> p d e", e=E),
                at[h][:, :DK].unsqueeze(2).broadcast_to((P, DK, E)),
                sc[h][:],
                mreps[(qt, h)][:].rearrange("p (d e) -> p d e", e=E),
                op0=OP.mult, op1=OP.mult)
            ch.append(c)
        cmerged = work.tile([P, DK * E], BF, tag="cmerged")
        nc.vector.tensor_tensor(cmerged[:], ch[0][:], ch[1][:], op=OP.add)
        # transpose c via PE (bf16) -> cT chunks; final matmul
        ps_o = ps_sm.tile([P, D], FP, tag="ps")
        for ci in range(DC):
            ps_ct = ps_sm.tile([P, P], BF, tag="ps")
            nc.tensor.matmul(ps_ct[:], cmerged[:, ci * P:(ci + 1) * P], ident_b[:],
                             is_transpose=True)
            cT = work.tile([P, P], BF, tag="cT")
            nc.vector.tensor_copy(cT[:], ps_ct[:])
            nc.tensor.matmul(ps_o[:], cT[:], w[("Wo_f", ci)][:],
                             start=(ci == 0), stop=False)
        nc.tensor.matmul(ps_o[:], combT[:, qs], bo[:], start=False, stop=True)
        o = work.tile([P, D], FP, tag="o")
        nc.scalar.activation(o[:], ps_o[:], AF.Copy)
        nc.sync.dma_start(out_d[qs, :], o[:])


_PROGRAM = None


def get_program():
    global _PROGRAM
    if _PROGRAM is None:
        nc = bacc.Bacc("TRN2", target_bir_lowering=False, debug=False,
                       enable_asserts=False, num_devices=8)
        from contextlib import ExitStack
        with tile.TileContext(nc) as tc, ExitStack() as ctx:
            _emit(nc, tc, ctx)
        nc.compile()
        _PROGRAM = nc
    return _PROGRAM


def make_in_maps(query, key, value, Wg, Wk, bk, Wv, bv, Wq, bq, Wo, bo):
    import ml_dtypes
    f32 = lambda x: np.ascontiguousarray(np.asarray(x), dtype=np.float32)
    bf = lambda x: np.ascontiguousarray(np.asarray(x), dtype=ml_dtypes.bfloat16)

    def hilo(x):
        x = np.asarray(x, np.float32)
        hi = x.astype(ml_dtypes.bfloat16)
        lo = (x - hi.astype(np.float32)).astype(ml_dtypes.bfloat16)
        return np.ascontiguousarray(hi), np.ascontiguousarray(lo)

    Wg_hi, Wg_lo = hilo(Wg)
    Wk2 = np.concatenate([np.asarray(Wk), np.asarray(Wk)], axis=1)  # [512, 128]
    bk2 = np.concatenate([np.asarray(bk), np.asarray(bk)])          # [128]
    shared = {
        "Wg_hi": Wg_hi, "Wg_lo": Wg_lo,
        "Wk2": bf(Wk2), "Wv": bf(Wv),
        # (d, e)-major expert columns
        "Wq_f": bf(np.asarray(Wq).transpose(1, 2, 0).reshape(D, DK * E)),
        "Wo_f": bf(np.asarray(Wo).transpose(1, 0, 2).reshape(DK * E, D)),
        "bkT2": f32(bk2.reshape(P, 1)),
        "bvT": f32(np.asarray(bv).reshape(DK, 1)),
        "bq_f": bf(np.asarray(bq).T.reshape(1, DK * E)),
        "bo": bf(bo),
        "ones1": np.ones((1, P), ml_dtypes.bfloat16),
        "onescol": np.ones((P, NKC), ml_dtypes.bfloat16),
    }
    in_maps = []
    for b in range(4):
        kT = bf(np.asarray(key[b]).T)
        vT = bf(np.asarray(value[b]).T)
        for h in range(2):
            qT_hi, qT_lo = hilo(np.asarray(query[b][h * NQ:(h + 1) * NQ, :]).T)
            in_maps.append({"kT": kT, "vT": vT,
                            "qT_hi": qT_hi, "qT_lo": qT_lo, **shared})
    return in_maps


def kernel(query, key, value, Wg, Wk, bk, Wv, bv, Wq, bq, Wo, bo):
    in_maps = make_in_maps(query, key, value, Wg, Wk, bk, Wv, bv, Wq, bq, Wo, bo)
    nc = get_program()
    res = bass_utils.run_bass_kernel_spmd(nc, in_maps, core_ids=list(range(8)))
    outs = [res.results[c]["out"] for c in range(8)]
    return np.concatenate(outs, axis=0).reshape(4, T, D).astype(np.float32)
